# revision 10
# baseline (speedup 1.0000x reference)
"""Conv2d(128->256, 3x3, pad 1, stride 1) on 32x56x56 fp32, for 8 trn2 cores.

Strategy: data-parallel over batch N=32 -> 4 images/core. Per core an
implicit-GEMM conv: C_in=128 is the partition (contraction) dim; for each
(kh, kw) tap a [128ci x 128co] weight tile multiplies a shifted window of the
zero-padded input image held in SBUF, accumulating into PSUM over the 9 taps.
Output rows are processed in chunks of 8 (free dim 8*56=448 <= 512 PSUM bank).
Matmuls run in float16 (inputs ~N(0,0.03..1): fp16 keeps ~2.6e-4 rel err) with fp32 PSUM accumulate; fp16 enables fast weight load so the 504 LDWEIGHTS hide under the matmul stream.

Rings: SP carries x input, ACT carries weights/bias + half-0 outputs,
SWDGE(Pool) carries half-1 outputs. Weights are laid out half-major on the
host so the half-0 weight DMA (the first-matmul gate) is small and lands
first; image-0's top rows are split into two DMAs for the same reason.
Staging pools use bufs=1 so later images' loads queue behind the cast that
frees the slot instead of competing with the critical head transfers.
"""
import numpy as np
from contextlib import ExitStack

N_FULL, C_IN, H, W = 32, 128, 56, 56
C_OUT, KS = 256, 3
N_CORES = 8
N_PER = N_FULL // N_CORES          # 4 images per core
HP = H + 2                          # 58 padded
PIX = H * W                         # 3136
ROWS = 8                            # output rows per psum chunk
RC = H // ROWS                      # 7 chunks
NF = ROWS * W                       # 448 free elems per matmul

T_ROWS = 34                         # xpad_top: padded rows 0..33  (chunks 0-3)
B_ROWS = 26                         # xpad_bot: padded rows 32..57 (chunks 4-6)
XT_R = 33                           # x rows 0..32 feed top interior
XT_A = 17                           # first sub-DMA: x rows 0..16
XT_B = XT_R - XT_A                  # second sub-DMA: x rows 17..32
XB_R = 25                           # x rows 31..55 feed bottom interior

_CACHE = {}


def _build():
    import concourse.tile as tile
    from concourse import mybir, bacc

    f32 = mybir.dt.float32
    f16 = mybir.dt.float16

    nc = bacc.Bacc("TRN2", target_bir_lowering=False, debug=False)
    x_d = nc.dram_tensor("x", [N_PER, C_IN, H, W], f16, kind="ExternalInput").ap()
    # host-pretransposed: [ci, half, k, co_half] (half-major, contiguous per half)
    w_d = nc.dram_tensor("w", [C_IN, 2, KS * KS, 128], f16, kind="ExternalInput").ap()
    b_d = nc.dram_tensor("b", [C_OUT], f32, kind="ExternalInput").ap()
    y_d = nc.dram_tensor("y", [N_PER, C_OUT, H, W], f32, kind="ExternalOutput").ap()

    with tile.TileContext(nc) as tc:
        with ExitStack() as ctx:
            wp = ctx.enter_context(tc.tile_pool(name="wp", bufs=1))
            xrawta = ctx.enter_context(tc.tile_pool(name="xrawta", bufs=1))
            xrawtb = ctx.enter_context(tc.tile_pool(name="xrawtb", bufs=1))
            xrawb = ctx.enter_context(tc.tile_pool(name="xrawb", bufs=1))
            xpadt = ctx.enter_context(tc.tile_pool(name="xpadt", bufs=2))
            xpadb = ctx.enter_context(tc.tile_pool(name="xpadb", bufs=2))
            pp = ctx.enter_context(tc.tile_pool(name="pp", bufs=4, space="PSUM"))
            op = ctx.enter_context(tc.tile_pool(name="op", bufs=2))

            # Weights (host-cast fp16) via ACT ring, half 0 first
            w_r = wp.tile([C_IN, 2 * KS * KS * 128], f16)
            w_r4 = w_r[:].rearrange("p (h k co) -> p h k co", h=2, k=KS * KS)
            for half in range(2):
                nc.scalar.dma_start(
                    w_r4[:, half], w_d[:, half].rearrange("ci k co -> ci (k co)")
                )

            # Bias: [256] -> [128 partitions, 2 halves]
            bias_sb = wp.tile([128, 2], f32)
            nc.scalar.dma_start(bias_sb[:], b_d.rearrange("(h p) -> p h", h=2))

            # PE warmup: ~3.4us of dummy matmuls while the head DMAs land, so
            # the HAM clock gate opens before the first real matmul issues.
            wu = wp.tile([128, 448], f16)
            nc.vector.memset(wu[:], 0.0)
            wups = pp.tile([128, NF], f32, tag="ps")
            for _ in range(8):
                nc.tensor.matmul(wups[:], wu[:, 0:128], wu[:], start=True, stop=True)

            for n in range(N_PER):
                # top interior in two slices so the first chunks unblock early
                xrta = xrawta.tile([C_IN, XT_A * W], f16)
                nc.sync.dma_start(xrta[:, : 9 * W], x_d[n, :, 0:9, :].rearrange("c h w -> c (h w)"))
                nc.scalar.dma_start(xrta[:, 9 * W :], x_d[n, :, 9:XT_A, :].rearrange("c h w -> c (h w)"))
                xrtb = xrawtb.tile([C_IN, XT_B * W], f16)
                nc.sync.dma_start(xrtb[:], x_d[n, :, XT_A:XT_R, :].rearrange("c h w -> c (h w)"))
                # bottom: x rows 31..55 -> padded rows 32..56 (local 0..24)
                xrb = xrawb.tile([C_IN, XB_R * W], f16)
                nc.sync.dma_start(xrb[:], x_d[n, :, 31 : 31 + XB_R, :].rearrange("c h w -> c (h w)"))

                xpt = xpadt.tile([C_IN, T_ROWS * HP], f16)
                xpt3 = xpt[:].rearrange("p (a b) -> p a b", a=T_ROWS)
                nc.vector.memset(xpt3[:, 0, :], 0.0)
                nc.vector.memset(xpt3[:, 1:T_ROWS, 0:1], 0.0)
                nc.vector.memset(xpt3[:, 1:T_ROWS, HP - 1 : HP], 0.0)
                nc.vector.tensor_copy(
                    xpt3[:, 1 : 1 + XT_A, 1 : 1 + W],
                    xrta[:].rearrange("p (a b) -> p a b", a=XT_A),
                )
                nc.vector.tensor_copy(
                    xpt3[:, 1 + XT_A : 1 + XT_R, 1 : 1 + W],
                    xrtb[:].rearrange("p (a b) -> p a b", a=XT_B),
                )

                xpb = xpadb.tile([C_IN, B_ROWS * HP], f16)
                xpb3 = xpb[:].rearrange("p (a b) -> p a b", a=B_ROWS)
                nc.vector.memset(xpb3[:, B_ROWS - 1, :], 0.0)
                nc.vector.memset(xpb3[:, 0 : B_ROWS - 1, 0:1], 0.0)
                nc.vector.memset(xpb3[:, 0 : B_ROWS - 1, HP - 1 : HP], 0.0)
                nc.vector.tensor_copy(
                    xpb3[:, 0 : B_ROWS - 1, 1 : 1 + W],
                    xrb[:].rearrange("p (a b) -> p a b", a=XB_R),
                )

                out_sb = op.tile([128, 2 * PIX], f32)
                last_img = n == N_PER - 1
                for half in range(2):
                    for rc in range(RC):
                        ps = pp.tile([128, NF], f32)
                        for kh in range(KS):
                            for kw in range(KS):
                                k = kh * KS + kw
                                lhsT = w_r4[:, half, k, :]
                                if rc < 4:
                                    rhs = xpt3[:, rc * ROWS + kh : rc * ROWS + kh + ROWS, kw : kw + W]
                                else:
                                    lr = (rc - 4) * ROWS + kh
                                    rhs = xpb3[:, lr : lr + ROWS, kw : kw + W]
                                nc.tensor.matmul(
                                    ps[:], lhsT, rhs,
                                    start=(k == 0), stop=(k == KS * KS - 1),
                                )
                        # psum -> sbuf with per-channel bias add
                        nc.vector.tensor_scalar_add(
                            out_sb[:, half * PIX + rc * NF : half * PIX + (rc + 1) * NF],
                            ps[:],
                            bias_sb[:, half : half + 1],
                        )
                        if last_img and half == 1:
                            # fine-grained tail: ship each chunk as it finishes
                            nc.gpsimd.dma_start(
                                y_d[n, 128:256, rc * ROWS : (rc + 1) * ROWS, :]
                                .rearrange("c h w -> c (h w)"),
                                out_sb[:, half * PIX + rc * NF : half * PIX + (rc + 1) * NF],
                            )
                    if not (last_img and half == 1):
                        eng = nc.scalar if half == 0 else nc.gpsimd
                        eng.dma_start(
                            y_d[n, half * 128 : (half + 1) * 128].rearrange("c h w -> c (h w)"),
                            out_sb[:, half * PIX : (half + 1) * PIX],
                        )
    nc.compile()
    return nc


def _get_nc():
    if "nc" not in _CACHE:
        _CACHE["nc"] = _build()
    return _CACHE["nc"]


def _prep_inputs(x, weight, bias):
    # fp16 on host: halves input DMA bytes and drops the on-device casts;
    # same rounding the device cast would apply
    x = np.ascontiguousarray(np.asarray(x, dtype=np.float32).astype(np.float16))
    # [co, ci, kh, kw] -> [ci, half, kh*kw, co_half], half-major so the half-0
    # block is contiguous and can be DMA'd first
    w_t = np.ascontiguousarray(
        np.transpose(np.asarray(weight, dtype=np.float32), (1, 2, 3, 0))
        .reshape(C_IN, KS * KS, 2, 128)
        .transpose(0, 2, 1, 3)
        .astype(np.float16)
    )
    b = np.ascontiguousarray(bias, dtype=np.float32)
    return x, w_t, b


def kernel(x, weight, bias):
    from concourse.bass_utils import run_bass_kernel_spmd

    x, w_t, b = _prep_inputs(x, weight, bias)
    nc = _get_nc()
    in_maps = [
        {"x": x[i * N_PER : (i + 1) * N_PER], "w": w_t, "b": b}
        for i in range(N_CORES)
    ]
    res = run_bass_kernel_spmd(nc, in_maps, list(range(N_CORES)))
    y = np.concatenate([res.results[i]["y"] for i in range(N_CORES)], axis=0)
    return y


# revision 11
# speedup vs baseline: 1.0783x; 1.0783x over previous
"""Conv2d(128->256, 3x3, pad 1, stride 1) on 32x56x56 fp32, for 8 trn2 cores.

Strategy: data-parallel over batch N=32 -> 4 images/core. Per core an
implicit-GEMM conv: C_in=128 is the partition (contraction) dim; for each
(kh, kw) tap a [128ci x 128co] weight tile multiplies a shifted window of the
zero-padded input image held in SBUF, accumulating into PSUM over the 9 taps.
Output rows are processed in chunks of 8 (free dim 8*56=448 <= 512 PSUM bank).
Matmuls run in float16 (inputs ~N(0,0.03..1): fp16 keeps ~2.6e-4 rel err) with fp32 PSUM accumulate; fp16 enables fast weight load so the 504 LDWEIGHTS hide under the matmul stream.

Rings: SP carries x input, ACT carries weights/bias + half-0 outputs,
SWDGE(Pool) carries half-1 outputs. Weights are laid out half-major on the
host so the half-0 weight DMA (the first-matmul gate) is small and lands
first; image-0's top rows are split into two DMAs for the same reason.
Staging pools use bufs=1 so later images' loads queue behind the cast that
frees the slot instead of competing with the critical head transfers.
"""
import numpy as np
from contextlib import ExitStack

N_FULL, C_IN, H, W = 32, 128, 56, 56
C_OUT, KS = 256, 3
N_CORES = 8
N_PER = N_FULL // N_CORES          # 4 images per core
HP = H + 2                          # 58 padded
PIX = H * W                         # 3136
ROWS = 8                            # output rows per psum chunk
RC = H // ROWS                      # 7 chunks
NF = ROWS * W                       # 448 free elems per matmul

T_ROWS = 34                         # xpad_top: padded rows 0..33  (chunks 0-3)
B_ROWS = 26                         # xpad_bot: padded rows 32..57 (chunks 4-6)
XT_R = 33                           # x rows 0..32 feed top interior
XT_A = 17                           # first sub-DMA: x rows 0..16
XT_B = XT_R - XT_A                  # second sub-DMA: x rows 17..32
XB_R = 25                           # x rows 31..55 feed bottom interior

_CACHE = {}


def _build():
    import concourse.tile as tile
    from concourse import mybir, bacc

    f32 = mybir.dt.float32
    f16 = mybir.dt.float16

    nc = bacc.Bacc("TRN2", target_bir_lowering=False, debug=False)
    x_d = nc.dram_tensor("x", [N_PER, C_IN, H, W], f16, kind="ExternalInput").ap()
    # host-pretransposed: [ci, half, k, co_half] (half-major, contiguous per half)
    w_d = nc.dram_tensor("w", [C_IN, 2, KS * KS, 128], f16, kind="ExternalInput").ap()
    b_d = nc.dram_tensor("b", [C_OUT], f32, kind="ExternalInput").ap()
    y_d = nc.dram_tensor("y", [N_PER, C_OUT, H, W], f32, kind="ExternalOutput").ap()

    with tile.TileContext(nc) as tc:
        with ExitStack() as ctx:
            wp = ctx.enter_context(tc.tile_pool(name="wp", bufs=1))
            xrawta = ctx.enter_context(tc.tile_pool(name="xrawta", bufs=1))
            xrawtb = ctx.enter_context(tc.tile_pool(name="xrawtb", bufs=1))
            xrawb = ctx.enter_context(tc.tile_pool(name="xrawb", bufs=1))
            xpadt = ctx.enter_context(tc.tile_pool(name="xpadt", bufs=2))
            xpadb = ctx.enter_context(tc.tile_pool(name="xpadb", bufs=2))
            pp = ctx.enter_context(tc.tile_pool(name="pp", bufs=4, space="PSUM"))
            op = ctx.enter_context(tc.tile_pool(name="op", bufs=2))

            # Weight half 0 first on the ACT ring: it gates the first matmul.
            # Half 1 and bias are issued after image-0's input DMAs so they
            # don't sit ahead of them in the ring FIFOs.
            w_r = wp.tile([C_IN, 2 * KS * KS * 128], f16)
            w_r4 = w_r[:].rearrange("p (h k co) -> p h k co", h=2, k=KS * KS)
            nc.scalar.dma_start(
                w_r4[:, 0], w_d[:, 0].rearrange("ci k co -> ci (k co)")
            )

            # PE warmup: ~3.4us of dummy matmuls while the head DMAs land, so
            # the HAM clock gate opens before the first real matmul issues.
            wu = wp.tile([128, 448], f16)
            nc.vector.memset(wu[:], 0.0)
            wups = pp.tile([128, NF], f32, tag="ps")
            for _ in range(9):
                nc.tensor.matmul(wups[:], wu[:, 0:128], wu[:], start=True, stop=True)

            bias_sb = wp.tile([128, 2], f32)

            for n in range(N_PER):
                # top interior in two slices so the first chunks unblock early
                xrta = xrawta.tile([C_IN, XT_A * W], f16)
                nc.sync.dma_start(xrta[:], x_d[n, :, 0:XT_A, :].rearrange("c h w -> c (h w)"))
                xrtb = xrawtb.tile([C_IN, XT_B * W], f16)
                nc.sync.dma_start(xrtb[:], x_d[n, :, XT_A:XT_R, :].rearrange("c h w -> c (h w)"))
                # bottom: x rows 31..55 -> padded rows 32..56 (local 0..24)
                xrb = xrawb.tile([C_IN, XB_R * W], f16)
                nc.sync.dma_start(xrb[:], x_d[n, :, 31 : 31 + XB_R, :].rearrange("c h w -> c (h w)"))

                if n == 0:
                    # now that image-0's loads are queued: weight half 1 + bias
                    nc.scalar.dma_start(
                        w_r4[:, 1], w_d[:, 1].rearrange("ci k co -> ci (k co)")
                    )
                    nc.scalar.dma_start(bias_sb[:], b_d.rearrange("(h p) -> p h", h=2))

                xpt = xpadt.tile([C_IN, T_ROWS * HP], f16)
                xpt3 = xpt[:].rearrange("p (a b) -> p a b", a=T_ROWS)
                nc.vector.memset(xpt3[:, 0, :], 0.0)
                nc.vector.memset(xpt3[:, 1:T_ROWS, 0:1], 0.0)
                nc.vector.memset(xpt3[:, 1:T_ROWS, HP - 1 : HP], 0.0)
                nc.vector.tensor_copy(
                    xpt3[:, 1 : 1 + XT_A, 1 : 1 + W],
                    xrta[:].rearrange("p (a b) -> p a b", a=XT_A),
                )
                nc.vector.tensor_copy(
                    xpt3[:, 1 + XT_A : 1 + XT_R, 1 : 1 + W],
                    xrtb[:].rearrange("p (a b) -> p a b", a=XT_B),
                )

                xpb = xpadb.tile([C_IN, B_ROWS * HP], f16)
                xpb3 = xpb[:].rearrange("p (a b) -> p a b", a=B_ROWS)
                nc.vector.memset(xpb3[:, B_ROWS - 1, :], 0.0)
                nc.vector.memset(xpb3[:, 0 : B_ROWS - 1, 0:1], 0.0)
                nc.vector.memset(xpb3[:, 0 : B_ROWS - 1, HP - 1 : HP], 0.0)
                nc.vector.tensor_copy(
                    xpb3[:, 0 : B_ROWS - 1, 1 : 1 + W],
                    xrb[:].rearrange("p (a b) -> p a b", a=XB_R),
                )

                out_sb = op.tile([128, 2 * PIX], f32)
                last_img = n == N_PER - 1
                for half in range(2):
                    for rc in range(RC):
                        ps = pp.tile([128, NF], f32)
                        for kh in range(KS):
                            for kw in range(KS):
                                k = kh * KS + kw
                                lhsT = w_r4[:, half, k, :]
                                if rc < 4:
                                    rhs = xpt3[:, rc * ROWS + kh : rc * ROWS + kh + ROWS, kw : kw + W]
                                else:
                                    lr = (rc - 4) * ROWS + kh
                                    rhs = xpb3[:, lr : lr + ROWS, kw : kw + W]
                                nc.tensor.matmul(
                                    ps[:], lhsT, rhs,
                                    start=(k == 0), stop=(k == KS * KS - 1),
                                )
                        # psum -> sbuf with per-channel bias add
                        nc.vector.tensor_scalar_add(
                            out_sb[:, half * PIX + rc * NF : half * PIX + (rc + 1) * NF],
                            ps[:],
                            bias_sb[:, half : half + 1],
                        )
                        if last_img and half == 1:
                            # fine-grained tail: ship each chunk as it finishes
                            nc.gpsimd.dma_start(
                                y_d[n, 128:256, rc * ROWS : (rc + 1) * ROWS, :]
                                .rearrange("c h w -> c (h w)"),
                                out_sb[:, half * PIX + rc * NF : half * PIX + (rc + 1) * NF],
                            )
                    if not (last_img and half == 1):
                        eng = nc.scalar if half == 0 else nc.gpsimd
                        eng.dma_start(
                            y_d[n, half * 128 : (half + 1) * 128].rearrange("c h w -> c (h w)"),
                            out_sb[:, half * PIX : (half + 1) * PIX],
                        )
    nc.compile()
    return nc


def _get_nc():
    if "nc" not in _CACHE:
        _CACHE["nc"] = _build()
    return _CACHE["nc"]


def _prep_inputs(x, weight, bias):
    # fp16 on host: halves input DMA bytes and drops the on-device casts;
    # same rounding the device cast would apply
    x = np.ascontiguousarray(np.asarray(x, dtype=np.float32).astype(np.float16))
    # [co, ci, kh, kw] -> [ci, half, kh*kw, co_half], half-major so the half-0
    # block is contiguous and can be DMA'd first
    w_t = np.ascontiguousarray(
        np.transpose(np.asarray(weight, dtype=np.float32), (1, 2, 3, 0))
        .reshape(C_IN, KS * KS, 2, 128)
        .transpose(0, 2, 1, 3)
        .astype(np.float16)
    )
    b = np.ascontiguousarray(bias, dtype=np.float32)
    return x, w_t, b


def kernel(x, weight, bias):
    from concourse.bass_utils import run_bass_kernel_spmd

    x, w_t, b = _prep_inputs(x, weight, bias)
    nc = _get_nc()
    in_maps = [
        {"x": x[i * N_PER : (i + 1) * N_PER], "w": w_t, "b": b}
        for i in range(N_CORES)
    ]
    res = run_bass_kernel_spmd(nc, in_maps, list(range(N_CORES)))
    y = np.concatenate([res.results[i]["y"] for i in range(N_CORES)], axis=0)
    return y
